# revision 8
# baseline (speedup 1.0000x reference)
import sys

import numpy as np

sys.path.insert(0, "/opt/trn_rl_repo")

import concourse.bass as bass
import concourse.mybir as mybir
from concourse.bass_utils import run_bass_kernel_spmd

# ROI max-pool: x (2048,256,256) f32 -> out (2048,7,7)
# out[c,i,j] = max(x[c, i:i+37, j:j+37]); only x[:, :43, :43] is ever read.
# V4: input DMA split by k-chunk so phase-1(k0) overlaps the k1 transfer;
# phase 1 = vertical sliding max (per-k), phase 2 = horizontal sliding max
# via the same suffix/prefix chain trick (replaces V2's 7 wide reduces).
# All compute on DVE; hazard-safe via V2-proven op sizing/ordering.

P = 128      # partitions
K = 2        # channels per partition
S = 43       # cropped spatial extent
W = 37       # pooling window
G = 7        # output grid
N_CORES = 8
FP32 = mybir.dt.float32

mx = mybir.AluOpType.max

_PROG = None


def _emit_p1(v, X, RD, k):
    """Vertical sliding max for channel-slab k: writes R rows 0..6 into
    RD[:,k,0:7,:]. D-chain staged in RD rows 8..13."""
    v.tensor_copy(out=RD[:, k : k + 1, 8, :], in_=X[:, k : k + 1, W, :])
    v.tensor_reduce(
        out=RD[:, k : k + 1, 6, :],
        in_=X[:, k : k + 1, G - 1 : W, :].transpose([0, 1, 3, 2]),
        axis=mybir.AxisListType.X,
        op=mx,
    )
    for s in range(1, 6):
        v.tensor_tensor(
            out=RD[:, k : k + 1, 6 - s : 9 + s : 2 + 2 * s, :],
            in0=RD[:, k : k + 1, 7 - s : 8 + s : 2 * s, :],
            in1=X[:, k : k + 1, 6 - s : 38 + s : 31 + 2 * s, :],
            op=mx,
        )
    v.tensor_tensor(
        out=RD[:, k : k + 1, 0, :],
        in0=RD[:, k : k + 1, 1, :],
        in1=X[:, k : k + 1, 0, :],
        op=mx,
    )
    return v.tensor_tensor(
        out=RD[:, k : k + 1, 1:7, :],
        in0=RD[:, k : k + 1, 1:7, :],
        in1=RD[:, k : k + 1, 8:14, :],
        op=mx,
    )


def _emit_p2(v, RD, CD):
    """Horizontal sliding max over columns for all 7 output rows and both k.
    Outputs land in CD[:,:,:,0:7]."""
    # reduce first: its fat runtime distances the copy's reads of RD col 37
    # from phase-1(k1)'s in-flight combine writes.
    v.tensor_reduce(
        out=CD[:, :, :, 6],
        in_=RD[:, :, 0:G, G - 1 : W],
        axis=mybir.AxisListType.X,
        op=mx,
    )
    v.tensor_copy(out=CD[:, :, :, 8], in_=RD[:, :, 0:G, W])
    # drain: the thin copy's stores must retire before s=1 reads slot 8
    # (~190ns dispatch gap < ~266ns retirement; p1 is safe only because its
    # fat reduce sits between the copy and the first chain step).
    v.drain()
    for s in range(1, 6):
        v.tensor_tensor(
            out=CD[:, :, :, 6 - s : 9 + s : 2 + 2 * s],
            in0=CD[:, :, :, 7 - s : 8 + s : 2 * s],
            in1=RD[:, :, 0:G, 6 - s : 38 + s : 31 + 2 * s],
            op=mx,
        )
    # A0: padded to a 2-slab op so in0 exactly matches step s=5's output AP
    # (rate-matched read order); the second lane writes junk to unused slot 15.
    v.tensor_tensor(
        out=CD[:, :, :, 0:16:15],
        in0=CD[:, :, :, 1:14:12],
        in1=RD[:, :, 0:G, 0:38:37],
        op=mx,
    )
    return v.tensor_tensor(
        out=CD[:, :, :, 1:7], in0=CD[:, :, :, 1:7], in1=CD[:, :, :, 8:14], op=mx
    )


def _build_program():
    nc = bass.Bass()
    x = nc.dram_tensor("x", [P, K, S, S], FP32, kind="ExternalInput")
    out_d = nc.dram_tensor("out", [P, K, G, G], FP32, kind="ExternalOutput")

    with (
        nc.Block() as block,
        nc.semaphore("sem_in0") as sem_in0,
        nc.semaphore("sem_in1") as sem_in1,
        nc.semaphore("sem_v2") as sem_v2,
        nc.semaphore("sem_out") as sem_out,
        nc.sbuf_tensor("X", [P, K, S, S], FP32) as X,
        nc.sbuf_tensor("RD", [P, K, 16, S], FP32) as RD,
        nc.sbuf_tensor("CD", [P, K, G, 16], FP32) as CD,
    ):

        @block.sync
        def _(sync):
            sync.dma_start(out=X[:, 0:1, :, :], in_=x[:, 0:1, :, :]).then_inc(
                sem_in0, 16
            )
            sync.dma_start(out=X[:, 1:2, :, :], in_=x[:, 1:2, :, :]).then_inc(
                sem_in1, 16
            )
            sync.wait_ge(sem_v2, 1)
            sync.dma_start(out=out_d[:, :, :, :], in_=CD[:, :, :, 0:G]).then_inc(
                sem_out, 16
            )
            sync.wait_ge(sem_out, 16)

        @block.vector
        def _(v):
            v.wait_ge(sem_in0, 16)
            _emit_p1(v, X, RD, 0)
            v.wait_ge(sem_in1, 16)
            _emit_p1(v, X, RD, 1)
            _emit_p2(v, RD, CD)
            # drain: make sure all trailing combine writes land in CD before
            # the out-DMA is released.
            v.drain().then_inc(sem_v2, 1)

    return nc


def kernel(x: np.ndarray) -> np.ndarray:
    global _PROG
    if _PROG is None:
        _PROG = _build_program()
    nc = _PROG

    x = np.asarray(x, dtype=np.float32)
    crop = np.ascontiguousarray(x[:, :S, :S])            # (2048, 43, 43)
    shards = crop.reshape(N_CORES, P, K, S, S)           # core-major channel split
    in_maps = [{"x": np.ascontiguousarray(shards[c])} for c in range(N_CORES)]

    res = run_bass_kernel_spmd(nc, in_maps, core_ids=list(range(N_CORES)))
    outs = [res.results[c]["out"].reshape(P * K, G, G) for c in range(N_CORES)]
    return np.concatenate(outs, axis=0).astype(np.float32)


# revision 9
# speedup vs baseline: 1.1155x; 1.1155x over previous
import sys

import numpy as np

sys.path.insert(0, "/opt/trn_rl_repo")

import concourse.bass as bass
import concourse.mybir as mybir
from concourse.bass_utils import run_bass_kernel_spmd

# ROI max-pool: x (2048,256,256) f32 -> out (2048,7,7)
# out[c,i,j] = max(x[c, i:i+37, j:j+37]); only x[:, :43, :43] is ever read.
# V4: input DMA split by k-chunk so phase-1(k0) overlaps the k1 transfer;
# phase 1 = vertical sliding max (per-k), phase 2 = horizontal sliding max
# via the same suffix/prefix chain trick (replaces V2's 7 wide reduces).
# All compute on DVE; hazard-safe via V2-proven op sizing/ordering.

P = 128      # partitions
K = 2        # channels per partition
S = 43       # cropped spatial extent
W = 37       # pooling window
G = 7        # output grid
N_CORES = 8
FP32 = mybir.dt.float32

mx = mybir.AluOpType.max

_PROG = None


def _emit_p1(v, X, RD, k):
    """Vertical sliding max for channel-slab k: writes R rows 0..6 into
    RD[:,k,0:7,:]. D-chain staged in RD rows 8..13."""
    v.tensor_copy(out=RD[:, k : k + 1, 8, :], in_=X[:, k : k + 1, W, :])
    v.tensor_reduce(
        out=RD[:, k : k + 1, 6, :],
        in_=X[:, k : k + 1, G - 1 : W, :].transpose([0, 1, 3, 2]),
        axis=mybir.AxisListType.X,
        op=mx,
    )
    for s in range(1, 6):
        v.tensor_tensor(
            out=RD[:, k : k + 1, 6 - s : 9 + s : 2 + 2 * s, :],
            in0=RD[:, k : k + 1, 7 - s : 8 + s : 2 * s, :],
            in1=X[:, k : k + 1, 6 - s : 38 + s : 31 + 2 * s, :],
            op=mx,
        )
    v.tensor_tensor(
        out=RD[:, k : k + 1, 0, :],
        in0=RD[:, k : k + 1, 1, :],
        in1=X[:, k : k + 1, 0, :],
        op=mx,
    )
    return v.tensor_tensor(
        out=RD[:, k : k + 1, 1:7, :],
        in0=RD[:, k : k + 1, 1:7, :],
        in1=RD[:, k : k + 1, 8:14, :],
        op=mx,
    )


def _emit_p2(v, RD, CD):
    """Horizontal sliding max over columns for all 7 output rows and both k.
    Outputs land in CD[:,:,:,0:7]."""
    # reduce first: its fat runtime distances the copy's reads of RD col 37
    # from phase-1(k1)'s in-flight combine writes.
    v.tensor_reduce(
        out=CD[:, :, :, 6],
        in_=RD[:, :, 0:G, G - 1 : W],
        axis=mybir.AxisListType.X,
        op=mx,
    )
    v.tensor_copy(out=CD[:, :, :, 8], in_=RD[:, :, 0:G, W])
    # drain: the thin copy's stores must retire before s=1 reads slot 8
    # (~190ns dispatch gap < ~266ns retirement; p1 is safe only because its
    # fat reduce sits between the copy and the first chain step).
    v.drain()
    for s in range(1, 6):
        v.tensor_tensor(
            out=CD[:, :, :, 6 - s : 9 + s : 2 + 2 * s],
            in0=CD[:, :, :, 7 - s : 8 + s : 2 * s],
            in1=RD[:, :, 0:G, 6 - s : 38 + s : 31 + 2 * s],
            op=mx,
        )
    # A0: padded to a 2-slab op so in0 exactly matches step s=5's output AP
    # (rate-matched read order); the second lane writes junk to unused slot 15.
    v.tensor_tensor(
        out=CD[:, :, :, 0:16:15],
        in0=CD[:, :, :, 1:14:12],
        in1=RD[:, :, 0:G, 0:38:37],
        op=mx,
    )
    return v.tensor_tensor(
        out=CD[:, :, :, 1:7], in0=CD[:, :, :, 1:7], in1=CD[:, :, :, 8:14], op=mx
    )


def _build_program():
    nc = bass.Bass()
    x = nc.dram_tensor("x", [P, K, S, S], FP32, kind="ExternalInput")
    out_d = nc.dram_tensor("out", [P, K, G, G], FP32, kind="ExternalOutput")

    with (
        nc.Block() as block,
        nc.semaphore("sem_in0") as sem_in0,
        nc.semaphore("sem_in1") as sem_in1,
        nc.semaphore("sem_v2") as sem_v2,
        nc.semaphore("sem_out") as sem_out,
        nc.sbuf_tensor("X", [P, K, S, S], FP32) as X,
        nc.sbuf_tensor("RD", [P, K, 16, S], FP32) as RD,
        nc.sbuf_tensor("CD", [P, K, G, 16], FP32) as CD,
    ):

        # input DMAs split across the two HWDGE queues (SP + Act) so each
        # k-slab transfers in half the time; k0 halves go first on both
        # queues so phase-1(k0) can start as early as possible.
        @block.sync
        def _(sync):
            sync.dma_start(out=X[:, 0:1, 0:22, :], in_=x[:, 0:1, 0:22, :]).then_inc(
                sem_in0, 16
            )
            sync.dma_start(out=X[:, 1:2, 0:22, :], in_=x[:, 1:2, 0:22, :]).then_inc(
                sem_in1, 16
            )
            sync.wait_ge(sem_v2, 1)
            sync.dma_start(out=out_d[:, :, :, :], in_=CD[:, :, :, 0:G]).then_inc(
                sem_out, 16
            )
            sync.wait_ge(sem_out, 16)

        @block.scalar
        def _(act):
            act.dma_start(out=X[:, 0:1, 22:43, :], in_=x[:, 0:1, 22:43, :]).then_inc(
                sem_in0, 16
            )
            act.dma_start(out=X[:, 1:2, 22:43, :], in_=x[:, 1:2, 22:43, :]).then_inc(
                sem_in1, 16
            )

        @block.vector
        def _(v):
            v.wait_ge(sem_in0, 32)
            _emit_p1(v, X, RD, 0)
            v.wait_ge(sem_in1, 32)
            _emit_p1(v, X, RD, 1)
            _emit_p2(v, RD, CD)
            # drain: make sure all trailing combine writes land in CD before
            # the out-DMA is released.
            v.drain().then_inc(sem_v2, 1)

    return nc


def kernel(x: np.ndarray) -> np.ndarray:
    global _PROG
    if _PROG is None:
        _PROG = _build_program()
    nc = _PROG

    x = np.asarray(x, dtype=np.float32)
    crop = np.ascontiguousarray(x[:, :S, :S])            # (2048, 43, 43)
    shards = crop.reshape(N_CORES, P, K, S, S)           # core-major channel split
    in_maps = [{"x": np.ascontiguousarray(shards[c])} for c in range(N_CORES)]

    res = run_bass_kernel_spmd(nc, in_maps, core_ids=list(range(N_CORES)))
    outs = [res.results[c]["out"].reshape(P * K, G, G) for c in range(N_CORES)]
    return np.concatenate(outs, axis=0).astype(np.float32)


# revision 10
# speedup vs baseline: 1.1518x; 1.0325x over previous
import sys

import numpy as np

sys.path.insert(0, "/opt/trn_rl_repo")

import concourse.bass as bass
import concourse.mybir as mybir
from concourse.bass_utils import run_bass_kernel_spmd

# ROI max-pool: x (2048,256,256) f32 -> out (2048,7,7)
# out[c,i,j] = max(x[c, i:i+37, j:j+37]); only x[:, :43, :43] is ever read.
# V4: input DMA split by k-chunk so phase-1(k0) overlaps the k1 transfer;
# phase 1 = vertical sliding max (per-k), phase 2 = horizontal sliding max
# via the same suffix/prefix chain trick (replaces V2's 7 wide reduces).
# All compute on DVE; hazard-safe via V2-proven op sizing/ordering.

P = 128      # partitions
K = 2        # channels per partition
S = 43       # cropped spatial extent
W = 37       # pooling window
G = 7        # output grid
N_CORES = 8
FP32 = mybir.dt.float32

mx = mybir.AluOpType.max

_PROG = None


def _emit_p1(v, X, RD, k):
    """Vertical sliding max for channel-slab k: writes R rows 0..6 into
    RD[:,k,0:7,:]. D-chain staged in RD rows 8..13."""
    v.tensor_copy(out=RD[:, k : k + 1, 8, :], in_=X[:, k : k + 1, W, :])
    v.tensor_reduce(
        out=RD[:, k : k + 1, 6, :],
        in_=X[:, k : k + 1, G - 1 : W, :].transpose([0, 1, 3, 2]),
        axis=mybir.AxisListType.X,
        op=mx,
    )
    for s in range(1, 6):
        v.tensor_tensor(
            out=RD[:, k : k + 1, 6 - s : 9 + s : 2 + 2 * s, :],
            in0=RD[:, k : k + 1, 7 - s : 8 + s : 2 * s, :],
            in1=X[:, k : k + 1, 6 - s : 38 + s : 31 + 2 * s, :],
            op=mx,
        )
    v.tensor_tensor(
        out=RD[:, k : k + 1, 0, :],
        in0=RD[:, k : k + 1, 1, :],
        in1=X[:, k : k + 1, 0, :],
        op=mx,
    )
    return v.tensor_tensor(
        out=RD[:, k : k + 1, 1:7, :],
        in0=RD[:, k : k + 1, 1:7, :],
        in1=RD[:, k : k + 1, 8:14, :],
        op=mx,
    )


def _emit_p2(v, RD, CD):
    """Horizontal sliding max over columns for all 7 output rows and both k.
    Outputs land in CD[:,:,:,0:7]."""
    # reduce first: its fat runtime distances the copy's reads of RD col 37
    # from phase-1(k1)'s in-flight combine writes.
    v.tensor_reduce(
        out=CD[:, :, :, 6],
        in_=RD[:, :, 0:G, G - 1 : W],
        axis=mybir.AxisListType.X,
        op=mx,
    )
    v.tensor_copy(out=CD[:, :, :, 8], in_=RD[:, :, 0:G, W])
    # drain: the thin copy's stores must retire before s=1 reads slot 8
    # (~190ns dispatch gap < ~266ns retirement; p1 is safe only because its
    # fat reduce sits between the copy and the first chain step).
    v.drain()
    for s in range(1, 6):
        v.tensor_tensor(
            out=CD[:, :, :, 6 - s : 9 + s : 2 + 2 * s],
            in0=CD[:, :, :, 7 - s : 8 + s : 2 * s],
            in1=RD[:, :, 0:G, 6 - s : 38 + s : 31 + 2 * s],
            op=mx,
        )
    # A0: padded to a 2-slab op so in0 exactly matches step s=5's output AP
    # (rate-matched read order); the second lane writes junk to unused slot 15.
    v.tensor_tensor(
        out=CD[:, :, :, 0:16:15],
        in0=CD[:, :, :, 1:14:12],
        in1=RD[:, :, 0:G, 0:38:37],
        op=mx,
    )
    return v.tensor_tensor(
        out=CD[:, :, :, 1:7], in0=CD[:, :, :, 1:7], in1=CD[:, :, :, 8:14], op=mx
    )


def _build_program():
    nc = bass.Bass()
    x = nc.dram_tensor("x", [P, K, S, S], FP32, kind="ExternalInput")
    out_d = nc.dram_tensor("out", [P, K, G, G], FP32, kind="ExternalOutput")

    with (
        nc.Block() as block,
        nc.semaphore("sem_in0") as sem_in0,
        nc.semaphore("sem_in1") as sem_in1,
        nc.semaphore("sem_in0p") as sem_in0p,
        nc.semaphore("sem_v2") as sem_v2,
        nc.semaphore("sem_out") as sem_out,
        nc.sbuf_tensor("X", [P, K, S, S], FP32) as X,
        nc.sbuf_tensor("RD", [P, K, 16, S], FP32) as RD,
        nc.sbuf_tensor("CD", [P, K, G, 16], FP32) as CD,
    ):

        # input DMAs split across the two HWDGE queues (SP + Act) so each
        # k-slab transfers in half the time; k0 halves go first on both
        # queues so phase-1(k0) can start as early as possible.
        @block.sync
        def _(sync):
            sync.dma_start(out=X[:, 0:1, 0:15, :], in_=x[:, 0:1, 0:15, :]).then_inc(
                sem_in0, 16
            )
            sync.dma_start(out=X[:, 1:2, 0:22, :], in_=x[:, 1:2, 0:22, :]).then_inc(
                sem_in1, 16
            )
            sync.wait_ge(sem_v2, 1)
            sync.dma_start(out=out_d[:, :, :, :], in_=CD[:, :, :, 0:G]).then_inc(
                sem_out, 16
            )
            sync.wait_ge(sem_out, 16)

        @block.scalar
        def _(act):
            act.dma_start(out=X[:, 0:1, 15:30, :], in_=x[:, 0:1, 15:30, :]).then_inc(
                sem_in0, 16
            )
            act.dma_start(out=X[:, 1:2, 22:43, :], in_=x[:, 1:2, 22:43, :]).then_inc(
                sem_in1, 16
            )

        @block.gpsimd
        def _(pool):
            pool.dma_start(out=X[:, 0:1, 30:43, :], in_=x[:, 0:1, 30:43, :]).then_inc(
                sem_in0p, 16
            )

        @block.vector
        def _(v):
            v.wait_ge(sem_in0, 32)
            v.wait_ge(sem_in0p, 16)
            _emit_p1(v, X, RD, 0)
            v.wait_ge(sem_in1, 32)
            _emit_p1(v, X, RD, 1)
            _emit_p2(v, RD, CD)
            # drain: make sure all trailing combine writes land in CD before
            # the out-DMA is released.
            v.drain().then_inc(sem_v2, 1)

    return nc


def kernel(x: np.ndarray) -> np.ndarray:
    global _PROG
    if _PROG is None:
        _PROG = _build_program()
    nc = _PROG

    x = np.asarray(x, dtype=np.float32)
    crop = np.ascontiguousarray(x[:, :S, :S])            # (2048, 43, 43)
    shards = crop.reshape(N_CORES, P, K, S, S)           # core-major channel split
    in_maps = [{"x": np.ascontiguousarray(shards[c])} for c in range(N_CORES)]

    res = run_bass_kernel_spmd(nc, in_maps, core_ids=list(range(N_CORES)))
    outs = [res.results[c]["out"].reshape(P * K, G, G) for c in range(N_CORES)]
    return np.concatenate(outs, axis=0).astype(np.float32)
